# revision 5
# baseline (speedup 1.0000x reference)
"""Stereo cost-volume generator (nn_CostGenerator) for 8 Trainium2 cores.

cost[b, c, d, h, w] = left[b, c, h, w] - right[b, c, h, w - d]  (0 where w < d)

Sharding: the 64 (B*C) channels are split 8-per-core (data parallel).

Per channel the 48 disparity slices are computed as 6 groups of 8
rows (e = 47-d, group k = rows e in [8k, 8k+8)) with ONE tensor_sub
per group using an overlapping (Hankel) access pattern on a zero-padded
right image: in1[i, t] = rpad[40 + i + t], against a broadcast left.
Each group is stored PACKED at its own width W_k = 216 + 8k (the widest
row of the group), which drops the all-zero wedge from both the
elementwise work and the DMA bytes (11328 instead of 12288 elems per
partition). The 168 garbage cells per channel (group-local row i, cols
[0, 7-i), where the Hankel window still overlaps the zero pad) are NOT
zeroed on device: the host unpack simply does not copy them (the valid
region of disparity d starts at w = d), so they land on the np.zeros
canvas like the rest of the skipped wedge.

Compute is split across engines: the DVE (2x 16-bit mode, ~0.52ns/elem)
does groups 0-4, the Pool/GpSimd engine does group 5 concurrently.
Each channel's packed tile is streamed out with 4 contiguous DMAs
alternating between the two HWDGE rings (Sync and Scalar) so both DMA
queues stream in parallel.

All on-device traffic is bfloat16: inputs are rounded to bf16 on the
host and the packed cost volume is stored as bf16, then upcast to f32
during the host unpack. This halves the dominant HBM write traffic
(the kernel is at the DMA roofline in f32) and doubles DVE throughput.
Accuracy: the subtract error is ~2^-9*(|l|+|r|) ~ 4e-3 relative to the
output scale, an order of magnitude inside the 2e-2 gate.
"""

import numpy as np

B, C, H, W, D = 2, 32, 128, 256, 48
NCORES = 8
CH = (B * C) // NCORES  # channels per core
PW = W + D - 1  # padded right row: 47 zeros + 256 values
NG = D // 8  # 6 groups of 8 e-rows
WK = [216 + 8 * k for k in range(NG)]  # group widths
GOFF = [0]
for k in range(NG):
    GOFF.append(GOFF[-1] + 8 * WK[k])
PACK = GOFF[-1]  # 11328 elems per partition
# out-DMA split points: (start, end, ring) with rings alternating so both
# HWDGE queues stream concurrently; earlier groups flush first.
OUT_SPLITS = [
    (GOFF[0], GOFF[2], "sync"),  # groups 0-1, ready first
    (GOFF[2], GOFF[4], "scalar"),  # groups 2-3
    (GOFF[4], GOFF[5], "sync"),  # group 4 (last DVE group)
    (GOFF[5], GOFF[6], "scalar"),  # group 5 (Pool engine's group)
]
POOL_GROUP = NG - 1  # widest group computed on GpSimd in parallel with DVE


def _cap(ap, base_off, part_pitch, dims):
    """Custom AP on ap's tensor at ap.offset+base_off; partition dim [pitch, H],
    free dims = list of (stride, size)."""
    import bass_rust

    return bass_rust.AP(
        tensor=ap.tensor,
        offset=ap.offset + base_off,
        ap=bass_rust.VecI64Pair([[part_pitch, H]] + [list(d) for d in dims]),
    )


def _build_nc():
    import concourse.bacc as bacc
    import concourse.mybir as mybir
    from concourse.tile import TileContext

    bf16 = mybir.dt.bfloat16
    nc = bacc.Bacc()
    inp = nc.declare_dram_parameter("inp", [2, CH, H, W], bf16, isOutput=False)
    out = nc.declare_dram_parameter("out", [CH, H, PACK], bf16, isOutput=True)

    with TileContext(nc) as tc:
        with tc.tile_pool(name="io", bufs=1) as pool:
            lt = pool.tile([H, CH * W], bf16, tag="lt", name="lt")
            rp = pool.tile([H, CH * PW], bf16, tag="rp", name="rp")
            obufs = [
                pool.tile([H, PACK], bf16, tag=f"ob{i}", name=f"ob{i}")
                for i in range(3)
            ]

            # zero the 47-col pad strips of all right channels (one 2D memset)
            nc.vector.memset(_cap(rp, 0, CH * PW, [(PW, CH), (1, D - 1)]), 0.0)

            # channel-0 inputs first so compute can start early; input loads
            # go on the Scalar HWDGE ring (idle at kernel start).
            nc.scalar.dma_start(out=lt[:, :W], in_=inp[0][0])
            nc.scalar.dma_start(
                out=_cap(rp, D - 1, CH * PW, [(1, W)]), in_=inp[1][0]
            )
            # remaining channels
            nc.scalar.dma_start(
                out=_cap(lt, W, CH * W, [(W, CH - 1), (1, W)]),
                in_=inp[0][1:].transpose([1, 0, 2]),
            )
            nc.scalar.dma_start(
                out=_cap(rp, PW + D - 1, CH * PW, [(PW, CH - 1), (1, W)]),
                in_=inp[1][1:].transpose([1, 0, 2]),
            )

            def sub(eng, ob, j, k):
                wk, w0 = WK[k], 40 - 8 * k
                # ob[h, G_k + i*wk + t] = left[h, w0+t] - rpad[h, 40+i+t]
                eng.tensor_sub(
                    out=_cap(ob, GOFF[k], PACK, [(wk, 8), (1, wk)]),
                    in0=_cap(lt, j * W + w0, CH * W, [(0, 8), (1, wk)]),
                    in1=_cap(rp, j * PW + 40, CH * PW, [(1, 8), (1, wk)]),
                )

            for j in range(CH):
                ob = obufs[j % 3]
                sub(nc.gpsimd, ob, j, POOL_GROUP)  # Pool runs widest group
                for k in range(NG - 1):
                    sub(nc.vector, ob, j, k)  # DVE (2x bf16) runs the rest
                for a, b, ring in OUT_SPLITS:
                    eng = nc.sync if ring == "sync" else nc.scalar
                    eng.dma_start(out=out[j][:, a:b], in_=ob[:, a:b])
    nc.finalize()
    return nc


def _shard_inputs(left_feature, right_feature):
    import ml_dtypes

    bf16 = ml_dtypes.bfloat16
    lf = np.asarray(left_feature, dtype=np.float32).astype(bf16).reshape(B * C, H, W)
    rf = np.asarray(right_feature, dtype=np.float32).astype(bf16).reshape(B * C, H, W)
    in_maps = []
    for i in range(NCORES):
        sl = slice(i * CH, (i + 1) * CH)
        in_maps.append({"inp": np.ascontiguousarray(np.stack([lf[sl], rf[sl]]))})
    return in_maps


def _unpack_core(arr):
    # arr: [CH, H, PACK] packed bf16 -> [CH, D, H, W] dense f32 (d-order).
    # Row i of group k holds disparity d = 47 - (8k+i); its first 7-i cells
    # are garbage (Hankel window overlapping the zero pad) and the valid
    # region of disparity d starts at w = d, so copy cols [7-i:] only.
    cost = np.zeros((arr.shape[0], D, H, W), np.float32)
    for k in range(NG):
        wk, w0 = WK[k], 40 - 8 * k
        blk = arr[:, :, GOFF[k] : GOFF[k + 1]].reshape(arr.shape[0], H, 8, wk)
        for i in range(8):
            d = D - 1 - (8 * k + i)
            s = max(0, 7 - i)
            cost[:, d, :, w0 + s :] = blk[:, :, i, s:]
    return cost


def _gather(results):
    parts = [_unpack_core(np.asarray(r["out"])) for r in results]
    cost = np.concatenate(parts, axis=0).reshape(B, C, D, H, W)
    return np.ascontiguousarray(cost)


def kernel(left_feature, right_feature, max_disp_at_scale):
    assert int(max_disp_at_scale) == D, max_disp_at_scale
    from concourse.bass_utils import run_bass_kernel_spmd

    nc = _build_nc()
    in_maps = _shard_inputs(left_feature, right_feature)
    res = run_bass_kernel_spmd(nc, in_maps, core_ids=list(range(NCORES)))
    return _gather(res.results)


# revision 9
# speedup vs baseline: 1.1472x; 1.1472x over previous
"""Stereo cost-volume generator (nn_CostGenerator) for 8 Trainium2 cores.

cost[b, c, d, h, w] = left[b, c, h, w] - right[b, c, h, w - d]  (0 where w < d)

Sharding: the 64 (B*C) channels are split 8-per-core (data parallel).

Per channel the 48 disparity slices are computed as 6 groups of 8
rows (e = 47-d, group k = rows e in [8k, 8k+8)) with ONE tensor_sub
per group using an overlapping (Hankel) access pattern on a zero-padded
right image: in1[i, t] = rpad[40 + i + t], against a broadcast left.
Each group is stored PACKED at its own width W_k = 216 + 8k (the widest
row of the group), which drops the all-zero wedge from both the
elementwise work and the DMA bytes (11328 instead of 12288 elems per
partition). The 168 garbage cells per channel (group-local row i, cols
[0, 7-i), where the Hankel window still overlaps the zero pad) are NOT
zeroed on device: the host unpack simply does not copy them (the valid
region of disparity d starts at w = d), so they land on the np.zeros
canvas like the rest of the skipped wedge.

All 6 groups run on the DVE in 2x 16-bit mode (~0.52ns/elem). The
GpSimd engine must stay idle: it shares SBUF read/write ports with the
DVE, and co-running a gpsimd tensor_sub slows DVE ops by ~1.5x.

The output stream is the hard wall: the 16 DMA engines of a core
sustain ~26.5 B/ns each (~424 GB/s aggregate, measured; shared across
all queues, so multi-queue adds no bandwidth). The packed bf16 volume
(23.2 MB/core) therefore needs ~55us. Output DMAs alternate between
the Sync and Scalar HWDGE rings, and channel 0's first DMA covers
group 0 only, so streaming starts as early as possible and stays
saturated to the end.

All on-device traffic is bfloat16: inputs are rounded to bf16 on the
host and the packed cost volume is stored as bf16, then upcast to f32
during the host unpack. This halves the dominant HBM write traffic
(the kernel is at the DMA roofline in f32) and doubles DVE throughput.
Accuracy: the subtract error is ~2^-9*(|l|+|r|) ~ 4e-3 relative to the
output scale, an order of magnitude inside the 2e-2 gate.
"""

import numpy as np

B, C, H, W, D = 2, 32, 128, 256, 48
NCORES = 8
CH = (B * C) // NCORES  # channels per core
PW = W + D - 1  # padded right row: 47 zeros + 256 values
NG = D // 8  # 6 groups of 8 e-rows
WK = [216 + 8 * k for k in range(NG)]  # group widths
GOFF = [0]
for k in range(NG):
    GOFF.append(GOFF[-1] + 8 * WK[k])
PACK = GOFF[-1]  # 11328 elems per partition
# out-DMA split points: (start, end, ring). Channel 0 streams group-by-group
# pairs so the queue starts draining ~2.5us earlier; later channels use two
# balanced DMAs. Rings alternate so issue costs spread over both engines.
SPLITS_CH0 = [
    (GOFF[0], GOFF[1], "sync"),  # group 0 alone: earliest possible start
    (GOFF[1], GOFF[2], "scalar"),
    (GOFF[2], GOFF[4], "sync"),
    (GOFF[4], GOFF[6], "scalar"),
]
SPLITS_REST = [
    (GOFF[0], GOFF[3], "sync"),  # groups 0-2
    (GOFF[3], GOFF[6], "scalar"),  # groups 3-5
]


def _cap(ap, base_off, part_pitch, dims):
    """Custom AP on ap's tensor at ap.offset+base_off; partition dim [pitch, H],
    free dims = list of (stride, size)."""
    import bass_rust

    return bass_rust.AP(
        tensor=ap.tensor,
        offset=ap.offset + base_off,
        ap=bass_rust.VecI64Pair([[part_pitch, H]] + [list(d) for d in dims]),
    )


def _build_nc():
    import concourse.bacc as bacc
    import concourse.mybir as mybir
    from concourse.tile import TileContext

    bf16 = mybir.dt.bfloat16
    nc = bacc.Bacc()
    inp = nc.declare_dram_parameter("inp", [2, CH, H, W], bf16, isOutput=False)
    out = nc.declare_dram_parameter("out", [CH, H, PACK], bf16, isOutput=True)

    with TileContext(nc) as tc:
        with tc.tile_pool(name="io", bufs=1) as pool:
            lt = pool.tile([H, CH * W], bf16, tag="lt", name="lt")
            rp = pool.tile([H, CH * PW], bf16, tag="rp", name="rp")
            obufs = [
                pool.tile([H, PACK], bf16, tag=f"ob{i}", name=f"ob{i}")
                for i in range(3)
            ]

            # zero the 47-col pad strips of all right channels (one 2D memset)
            nc.vector.memset(_cap(rp, 0, CH * PW, [(PW, CH), (1, D - 1)]), 0.0)

            # channel-0 inputs first so compute can start early; they go on
            # the Sync ring (first engine out of the preamble) while the
            # remaining channels load via the Scalar ring concurrently.
            nc.sync.dma_start(out=lt[:, :W], in_=inp[0][0])
            nc.sync.dma_start(
                out=_cap(rp, D - 1, CH * PW, [(1, W)]), in_=inp[1][0]
            )
            # remaining channels
            nc.scalar.dma_start(
                out=_cap(lt, W, CH * W, [(W, CH - 1), (1, W)]),
                in_=inp[0][1:].transpose([1, 0, 2]),
            )
            nc.scalar.dma_start(
                out=_cap(rp, PW + D - 1, CH * PW, [(PW, CH - 1), (1, W)]),
                in_=inp[1][1:].transpose([1, 0, 2]),
            )

            def sub(eng, ob, j, k):
                wk, w0 = WK[k], 40 - 8 * k
                # ob[h, G_k + i*wk + t] = left[h, w0+t] - rpad[h, 40+i+t]
                eng.tensor_sub(
                    out=_cap(ob, GOFF[k], PACK, [(wk, 8), (1, wk)]),
                    in0=_cap(lt, j * W + w0, CH * W, [(0, 8), (1, wk)]),
                    in1=_cap(rp, j * PW + 40, CH * PW, [(1, 8), (1, wk)]),
                )

            for j in range(CH):
                ob = obufs[j % 3]
                for k in range(NG):
                    sub(nc.vector, ob, j, k)
                for a, b, ring in SPLITS_CH0 if j == 0 else SPLITS_REST:
                    eng = nc.sync if ring == "sync" else nc.scalar
                    eng.dma_start(out=out[j][:, a:b], in_=ob[:, a:b])
    nc.finalize()
    return nc


def _shard_inputs(left_feature, right_feature):
    import ml_dtypes

    bf16 = ml_dtypes.bfloat16
    lf = np.asarray(left_feature, dtype=np.float32).astype(bf16).reshape(B * C, H, W)
    rf = np.asarray(right_feature, dtype=np.float32).astype(bf16).reshape(B * C, H, W)
    in_maps = []
    for i in range(NCORES):
        sl = slice(i * CH, (i + 1) * CH)
        in_maps.append({"inp": np.ascontiguousarray(np.stack([lf[sl], rf[sl]]))})
    return in_maps


def _unpack_core(arr):
    # arr: [CH, H, PACK] packed bf16 -> [CH, D, H, W] dense f32 (d-order).
    # Row i of group k holds disparity d = 47 - (8k+i); its first 7-i cells
    # are garbage (Hankel window overlapping the zero pad) and the valid
    # region of disparity d starts at w = d, so copy cols [7-i:] only.
    cost = np.zeros((arr.shape[0], D, H, W), np.float32)
    for k in range(NG):
        wk, w0 = WK[k], 40 - 8 * k
        blk = arr[:, :, GOFF[k] : GOFF[k + 1]].reshape(arr.shape[0], H, 8, wk)
        for i in range(8):
            d = D - 1 - (8 * k + i)
            s = max(0, 7 - i)
            cost[:, d, :, w0 + s :] = blk[:, :, i, s:]
    return cost


def _gather(results):
    parts = [_unpack_core(np.asarray(r["out"])) for r in results]
    cost = np.concatenate(parts, axis=0).reshape(B, C, D, H, W)
    return np.ascontiguousarray(cost)


def kernel(left_feature, right_feature, max_disp_at_scale):
    assert int(max_disp_at_scale) == D, max_disp_at_scale
    from concourse.bass_utils import run_bass_kernel_spmd

    nc = _build_nc()
    in_maps = _shard_inputs(left_feature, right_feature)
    res = run_bass_kernel_spmd(nc, in_maps, core_ids=list(range(NCORES)))
    return _gather(res.results)
